# Initial kernel scaffold
#
"""Trainium2 Bass kernel for the lipsnet CustomModel problem.

Math: the reference computes, per sample,
    jac_norm = ||D3 W3 D2 W2 D1 W1||_F      (Di = diag(relu'(pi)))
    out = tanh(k_out * f_out / (jac_norm + 1e-4))
Key identity used here:  with G = W1 W1^T = L L^T (host eigen factorization),
    ||D3 W3 D2 W2 D1 W1||_F^2 = ||D3 W3 D2 W2 D1 L||_F^2
                              = sum_c || D3 W3 D2 (M_c @ d1) ||^2
where M_c[j,l] = W2[j,l] * L[l,c] are 85 host-precomputed stationary
matrices and d1/d2/d3 are the per-sample binary relu masks.  Every
per-sample 85x85x85 contraction becomes a stationary-weight matmul with
the mask tensor [85, S] as the moving operand, so the TensorEngine does
all the heavy lifting; the only full-size elementwise work per c is one
DVE mask-multiply and one ACT square.  The sum over c of squares is
accumulated on the TensorEngine itself via an identity-matmul into a
persistent PSUM tile.

Sharding: pure data parallel over the batch dim, 8 NeuronCores, weights
replicated.  kernel() takes FULL inputs and returns the FULL output.
"""

import os
from contextlib import ExitStack

import numpy as np

import concourse.bass as bass
import concourse.bacc as bacc
import concourse.mybir as mybir
import concourse.tile as tile

F32 = mybir.dt.float32
AF = mybir.ActivationFunctionType
OP = mybir.AluOpType

B = 8192
OBS = 64
ACTD = 16
H = 128
COMP = 85
KS = 32
NCORES = 8
S = B // NCORES        # 1024 samples per core
NB = S // 128          # 8 sample blocks of 128
CH = 512               # matmul moving-operand chunk (one PSUM bank of f32)
EPS = 1e-4

# name -> (shape, bf16?) of every replicated weight, packed host-side into
# two [128, N] arrays (one f32, one bf16) so the kernel needs just 2 DMAs
_WSLOTS = {
    "ow1T": ([OBS, H], 0), "ob1": ([H, 1], 0), "ow2T": ([H, H], 0),
    "ob2": ([H, 1], 0), "aw1T": ([ACTD, H], 0), "ab1": ([H, 1], 0),
    "aw2T": ([H, H], 0), "ab2": ([H, 1], 0),
    "kw1Ta": ([H, KS], 0), "kw1Tb": ([H, KS], 0), "kb1": ([KS, 1], 0),
    "kw2T": ([KS, KS // 2], 0), "kb2": ([KS // 2, 1], 0),
    "kw3T": ([KS // 2, 1], 0), "kb3": ([1, 1], 0),
    "mw1Ta": ([H, COMP], 0), "mw1Tb": ([H, COMP], 0), "mb1": ([COMP, 1], 0),
    "mw2T": ([COMP, COMP], 0), "mb2": ([COMP, 1], 0),
    "mw3T": ([COMP, COMP], 0), "mb3": ([COMP, 1], 0),
    "ones": ([COMP, 1], 0), "iden": ([H, H], 0),
    "mall": ([COMP, COMP * COMP], 1), "mw3Tb": ([COMP, COMP], 1),
    "idenb": ([COMP, COMP], 1), "onesb": ([COMP, 1], 1),
}
_OFFS = {}
_NCOLS = [0, 0]
for _n, (_shp, _b) in _WSLOTS.items():
    _OFFS[_n] = _NCOLS[_b]
    _NCOLS[_b] += _shp[1]

def host_prep(inputs):
    """Host-side weight preprocessing + packing (pure numpy, all tiny)."""
    import ml_dtypes
    f = lambda a: np.ascontiguousarray(np.asarray(a, dtype=np.float32))
    W1, W2, W3 = f(inputs["mw1"]), f(inputs["mw2"]), f(inputs["mw3"])
    G = (W1 @ W1.T).astype(np.float64)
    lam, U = np.linalg.eigh(G)
    L = (U * np.sqrt(np.clip(lam, 0.0, None))).astype(np.float32)  # G = L L^T
    # mall[l, c*85+j] = W2[j, l] * L[l, c]   (stage-1 stationary lhsT per c)
    mall = (W2.T[:, None, :] * L[:, :, None]).reshape(COMP, COMP * COMP)
    vals = {
        "ow1T": f(inputs["ow1"]).T, "ob1": f(inputs["ob1"]).reshape(H, 1),
        "ow2T": f(inputs["ow2"]).T, "ob2": f(inputs["ob2"]).reshape(H, 1),
        "aw1T": f(inputs["aw1"]).T, "ab1": f(inputs["ab1"]).reshape(H, 1),
        "aw2T": f(inputs["aw2"]).T, "ab2": f(inputs["ab2"]).reshape(H, 1),
        "kw1Ta": f(inputs["kw1"]).T[:H], "kw1Tb": f(inputs["kw1"]).T[H:],
        "kb1": f(inputs["kb1"]).reshape(KS, 1),
        "kw2T": f(inputs["kw2"]).T, "kb2": f(inputs["kb2"]).reshape(KS // 2, 1),
        "kw3T": f(inputs["kw3"]).T, "kb3": f(inputs["kb3"]).reshape(1, 1),
        "mw1Ta": W1.T[:H], "mw1Tb": W1.T[H:],
        "mb1": f(inputs["mb1"]).reshape(COMP, 1),
        "mw2T": W2.T, "mb2": f(inputs["mb2"]).reshape(COMP, 1),
        "mw3T": W3.T, "mb3": f(inputs["mb3"]).reshape(COMP, 1),
        "ones": np.ones((COMP, 1), np.float32),
        "iden": np.eye(H, dtype=np.float32),
        "mall": mall, "mw3Tb": W3.T,
        "idenb": np.eye(COMP, dtype=np.float32),
        "onesb": np.ones((COMP, 1), np.float32),
    }
    packs = [np.zeros((128, _NCOLS[0]), np.float32),
             np.zeros((128, _NCOLS[1]), ml_dtypes.bfloat16)]
    for n, (shp, b) in _WSLOTS.items():
        o = _OFFS[n]
        packs[b][:shp[0], o:o + shp[1]] = vals[n]
    return {"wpack32": packs[0], "wpack16": packs[1]}


def build_nc(reps=1):
    nc = bacc.Bacc()

    obs_d = nc.declare_dram_parameter("obs", [S, OBS], F32, isOutput=False)
    act_d = nc.declare_dram_parameter("action", [S, ACTD], F32, isOutput=False)
    BF16 = mybir.dt.bfloat16
    wp32_d = nc.declare_dram_parameter("wpack32", [128, _NCOLS[0]], F32,
                                       isOutput=False)
    wp16_d = nc.declare_dram_parameter("wpack16", [128, _NCOLS[1]], BF16,
                                       isOutput=False)
    tick_d = nc.declare_dram_parameter("tick", [1, 1], F32, isOutput=False)
    out_d = nc.declare_dram_parameter("out", [S, COMP], F32, isOutput=True)

    with tile.TileContext(nc) as tc, ExitStack() as ctx:
        wp = ctx.enter_context(tc.tile_pool(name="weights", bufs=1))
        ap = ctx.enter_context(tc.tile_pool(name="acts", bufs=1))
        zp = ctx.enter_context(tc.tile_pool(name="zbuf", bufs=4))
        sqp = ctx.enter_context(tc.tile_pool(name="sqbuf", bufs=4))
        outp = ctx.enter_context(tc.tile_pool(name="outbuf", bufs=3))
        smp = ctx.enter_context(tc.tile_pool(name="small", bufs=16))
        psA = ctx.enter_context(tc.tile_pool(name="psA", bufs=3, space="PSUM"))
        psC = ctx.enter_context(tc.tile_pool(name="psC", bufs=1, space="PSUM"))

        # ---- load weights (2 packed DMAs), expose per-weight slice views ----
        wp32 = wp.tile([128, _NCOLS[0]], F32, tag="wp32", name="wp32")
        wp16 = wp.tile([128, _NCOLS[1]], BF16, tag="wp16", name="wp16")
        nc.sync.dma_start(wp32[:], wp32_d[:])
        w = {}
        for name, (shp, b) in _WSLOTS.items():
            o = _OFFS[name]
            w[name] = (wp16 if b else wp32)[0:shp[0], o:o + shp[1]]

        tick_sb = wp.tile([1, 1], F32, tag="tick_sb", name="tick_sb")
        nc.sync.dma_start(tick_sb[:], tick_d[:])

        # ---- load + transpose obs/action into [feat, S] layout ----
        for _rep in range(reps):
            obs_sb = ap.tile([128, NB, OBS], F32, tag="obs_sb")
            act_sb = ap.tile([128, NB, ACTD], F32, tag="act_sb")
            for nb in range(NB):
                nc.sync.dma_start(obs_sb[:, nb, :], obs_d[nb * 128:(nb + 1) * 128, :])
                nc.sync.dma_start(act_sb[:, nb, :], act_d[nb * 128:(nb + 1) * 128, :])
            # collapse the many DMA-queue semaphores into one barrier so no
            # matmul needs more than one sync wait (walrus S3_LW limit)
            tc.strict_bb_all_engine_barrier()
            # the big bf16 pack (stage-1 matrices) is only needed at J-loop
            # start; issued after the barrier so the forward overlaps it
            nc.sync.dma_start(wp16[:], wp16_d[:])

            obst = ap.tile([OBS, S], F32, tag="obst")
            actt = ap.tile([ACTD, S], F32, tag="actt")
            for nb in range(NB):
                pt = psA.tile([OBS, 128], F32, tag="a")
                nc.tensor.transpose(pt[:], obs_sb[:, nb, :], w["iden"][:])
                nc.vector.tensor_copy(obst[:, nb * 128:(nb + 1) * 128], pt[:])
                pt2 = psA.tile([ACTD, 128], F32, tag="a")
                nc.tensor.transpose(pt2[:], act_sb[:, nb, :], w["iden"][:])
                nc.vector.tensor_copy(actt[:, nb * 128:(nb + 1) * 128], pt2[:])

            # ---- forward layers ([feat, S], chunked matmuls + fused ACT) ----
            def layer(dst, dst_sl, terms, bias, func, p):
                # dst[dst_sl] = func(sum_i lhsT_i.T @ rhs_i + bias), chunked over S
                m = dst.shape[-1] if dst_sl is None else None
                for ch in range(S // CH):
                    sl = slice(ch * CH, (ch + 1) * CH)
                    pt = p.tile([terms[0][0].shape[-1], CH], F32, tag="a", name="pt")
                    n = len(terms)
                    for i, (lhsT, rhs) in enumerate(terms):
                        nc.tensor.matmul(pt[:], lhsT[:], rhs[:, sl],
                                         start=(i == 0), stop=(i == n - 1))
                    dsl = dst[:, sl] if dst_sl is None else dst[dst_sl, sl]
                    if func == AF.Relu:
                        nc.vector.tensor_scalar(out=dsl, in0=pt[:], scalar1=bias[:],
                                                scalar2=0.0, op0=OP.add, op1=OP.max)
                    else:
                        nc.scalar.activation(dsl, pt[:], func, bias=bias[:])

            oh1 = ap.tile([H, S], F32, tag="oh1")
            layer(oh1, None, [(w["ow1T"], obst)], w["ob1"], AF.Relu, psA)
            of = ap.tile([H, S], F32, tag="of")
            layer(of, None, [(w["ow2T"], oh1)], w["ob2"], AF.Relu, psA)
            ah1 = ap.tile([H, S], F32, tag="ah1")
            layer(ah1, None, [(w["aw1T"], actt)], w["ab1"], AF.Relu, psA)
            af = ap.tile([H, S], F32, tag="af")
            layer(af, None, [(w["aw2T"], ah1)], w["ab2"], AF.Relu, psA)

            k1 = ap.tile([KS, S], F32, tag="k1")
            layer(k1, None, [(w["kw1Ta"], of), (w["kw1Tb"], af)], w["kb1"], AF.Tanh, psA)
            k2 = ap.tile([KS // 2, S], F32, tag="k2")
            layer(k2, None, [(w["kw2T"], k1)], w["kb2"], AF.Tanh, psA)

            # k_out = softplus(kw3 @ k2 + kb3) = ln(1 + exp(.)) via Exp then Ln(x+1)
            kexp = ap.tile([1, S], F32, tag="kexp")
            layer(kexp, None, [(w["kw3T"], k2)], w["kb3"], AF.Exp, psA)
            kout = ap.tile([1, S], F32, tag="kout")
            nc.scalar.activation(kout[:], kexp[:], AF.Ln, bias=1.0)

            h1 = ap.tile([COMP, S], F32, tag="h1")
            layer(h1, None, [(w["mw1Ta"], of), (w["mw1Tb"], af)], w["mb1"], AF.Relu, psA)
            d1 = ap.tile([COMP, S], BF16, tag="d1")
            nc.vector.tensor_scalar(out=d1[:], in0=h1[:], scalar1=0.0, scalar2=None,
                                    op0=OP.is_gt)
            h2 = ap.tile([COMP, S], F32, tag="h2")
            layer(h2, None, [(w["mw2T"], h1)], w["mb2"], AF.Relu, psA)
            d2 = ap.tile([COMP, S], F32, tag="d2")
            nc.vector.tensor_scalar(out=d2[:], in0=h2[:], scalar1=0.0, scalar2=None,
                                    op0=OP.is_gt)
            fout = ap.tile([COMP, S], F32, tag="fout")
            layer(fout, None, [(w["mw3T"], h2)], w["mb3"], AF.Relu, psA)
            d3 = ap.tile([COMP, S], F32, tag="d3")
            nc.vector.tensor_scalar(out=d3[:], in0=fout[:], scalar1=0.0,
                                    scalar2=None, op0=OP.is_gt)

            # ---- Jacobian-norm loop over the 85 columns of L ----
            # bf16 identity for the accumulate-matmul (fp32 matmuls lower to
            # HI/LO pairs that break inside an interleaved accumulation group)
            idenb = w["idenb"]
            accp = psC.tile([COMP, S], F32, tag="c")   # persistent PSUM accumulator
            ACCs = ap.tile([COMP, S], F32, tag="ACCs")  # SBUF spill of acc groups
            GRP = 28   # accumulation-group length (bounded for HW robustness)
            acc_n = [0]

            def acc_mm(sq):
                n = acc_n[0]
                for ch in range(S // CH):
                    sl = slice(ch * CH, (ch + 1) * CH)
                    nc.tensor.matmul(accp[:, sl], idenb[:], sq[:, sl],
                                     start=(n % GRP == 0),
                                     stop=(n % GRP == GRP - 1 or n == COMP - 1),
                                     skip_group_check=True)
                acc_n[0] = n + 1
                if n % GRP == GRP - 1 or n == COMP - 1:
                    if n < GRP:
                        nc.vector.tensor_copy(ACCs[:], accp[:])
                    else:
                        nc.vector.tensor_tensor(ACCs[:], accp[:], ACCs[:], OP.add)

            tc.strict_bb_all_engine_barrier()
            # software pipeline: py prefetched one c ahead of the DVE mask,
            # squares accumulated two c behind, so PE never heads-of-line
            # blocks the mask -> pr -> py -> mask cycle
            pys = {}

            def emit_py(c):
                t = psA.tile([COMP, S], F32, tag="a", name="py")
                for ch in range(S // CH):
                    sl = slice(ch * CH, (ch + 1) * CH)
                    nc.tensor.matmul(t[:, sl], w["mall"][:, c * COMP:(c + 1) * COMP],
                                     d1[:, sl], start=True, stop=True)
                pys[c] = t

            emit_py(0)
            pend = []
            for c in range(COMP):
                z = zp.tile([COMP, S], BF16, tag="z")
                nc.vector.tensor_tensor(z[:], pys.pop(c)[:], d2[:], OP.mult)
                if c + 1 < COMP:
                    emit_py(c + 1)
                if len(pend) == 2:
                    acc_mm(pend.pop(0))
                pr = psA.tile([COMP, S], F32, tag="a", name="pr")
                for ch in range(S // CH):
                    sl = slice(ch * CH, (ch + 1) * CH)
                    nc.tensor.matmul(pr[:, sl], w["mw3Tb"][:], z[:, sl],
                                     start=True, stop=True)
                sq = sqp.tile([COMP, S], BF16, tag="sq")
                nc.scalar.square(sq[:], pr[:])
                pend.append(sq)
            acc_mm(pend.pop(0))
            acc_mm(pend.pop(0))

            # ---- finale: jn2 = ones^T (d3 * acc); out = tanh(kout*fout/(sqrt+eps)) ----
            am = zp.tile([COMP, S], BF16, tag="am")
            nc.vector.tensor_tensor(am[:], ACCs[:], d3[:], OP.mult)
            pj = psA.tile([1, S], F32, tag="a", name="pj")
            pj_lhs = w["onesb"]
            for ch in range(S // CH):
                sl = slice(ch * CH, (ch + 1) * CH)
                nc.tensor.matmul(pj[:, sl], pj_lhs[:], am[:, sl],
                                 start=True, stop=True)
            jn2 = ap.tile([1, S], F32, tag="jn2")
            nc.scalar.copy(jn2[:], pj[:])

            tc.strict_bb_all_engine_barrier()

            # batch the per-sample scale: transpose jn2/kout for all blocks
            # into one [128, 2*NB] tile, then one sqrt + vector recip pass
            pjk = psA.tile([128, 2 * NB], F32, tag="a", name="pjk")
            for nb in range(NB):
                sl = slice(nb * 128, (nb + 1) * 128)
                nc.tensor.transpose(pjk[:, nb:nb + 1], jn2[:, sl], w["iden"][:1, :1])
                nc.tensor.transpose(pjk[:, NB + nb:NB + nb + 1], kout[:, sl],
                                    w["iden"][:1, :1])
            den = smp.tile([128, NB], F32, tag="den")
            nc.scalar.activation(den[:], pjk[:, 0:NB], AF.Sqrt)
            rec = smp.tile([128, NB], F32, tag="rec")
            nc.vector.tensor_scalar_add(rec[:], den[:], EPS)
            nc.vector.reciprocal(rec[:], rec[:])
            scl = smp.tile([128, NB], F32, tag="scl")
            nc.vector.tensor_tensor(scl[:], rec[:], pjk[:, NB:2 * NB], OP.mult)
            for nb in range(NB):
                sl = slice(nb * 128, (nb + 1) * 128)
                pt = psA.tile([128, COMP], F32, tag="a", name="ptf")
                nc.tensor.transpose(pt[:], fout[:, sl], w["iden"][:COMP, :COMP])
                ot = outp.tile([128, COMP], F32, tag="ot")
                nc.scalar.activation(ot[:], pt[:], AF.Tanh, scale=scl[:, nb:nb + 1])
                nc.sync.dma_start(out_d[sl, :], ot[:])

    return nc


_NC = None


def _get_nc():
    global _NC
    if _NC is None:
        _NC = build_nc()
        _NC.finalize()
    return _NC


def make_in_maps(inputs):
    w = host_prep(inputs)
    obs = np.ascontiguousarray(np.asarray(inputs["obs"], np.float32))
    act = np.ascontiguousarray(np.asarray(inputs["action"], np.float32))
    in_maps = []
    for i in range(NCORES):
        m = dict(w)
        m["obs"] = np.ascontiguousarray(obs[i * S:(i + 1) * S])
        m["action"] = np.ascontiguousarray(act[i * S:(i + 1) * S])
        m["tick"] = np.zeros((1, 1), np.float32)
        in_maps.append(m)
    return in_maps


def kernel(**inputs):
    from concourse.bass_utils import run_bass_kernel_spmd

    nc = _get_nc()
    in_maps = make_in_maps(inputs)
    res = run_bass_kernel_spmd(nc, in_maps, core_ids=list(range(NCORES)))
    return np.concatenate([r["out"] for r in res.results], axis=0)



# revision 1
# speedup vs baseline: 2.4581x; 2.4581x over previous
"""Trainium2 Bass kernel for the lipsnet CustomModel problem.

Math: the reference computes, per sample,
    jac_norm = ||D3 W3 D2 W2 D1 W1||_F      (Di = diag(relu'(pi)))
    out = tanh(k_out * f_out / (jac_norm + 1e-4))
Key identity used here:  with G = W1 W1^T = L L^T (host eigen factorization),
    ||D3 W3 D2 W2 D1 W1||_F^2 = ||D3 W3 D2 W2 D1 L||_F^2
                              = sum_c || D3 W3 D2 (M_c @ d1) ||^2
where M_c[j,l] = W2[j,l] * L[l,c] are 85 host-precomputed stationary
matrices and d1/d2/d3 are the per-sample binary relu masks.  Every
per-sample 85x85x85 contraction becomes a stationary-weight matmul with
the mask tensor [85, S] as the moving operand, so the TensorEngine does
all the heavy lifting; the only full-size elementwise work per c is one
DVE mask-multiply and one ACT square.  The sum over c of squares is
accumulated on the TensorEngine itself via an identity-matmul into a
persistent PSUM tile.

Sharding: pure data parallel over the batch dim, 8 NeuronCores, weights
replicated.  kernel() takes FULL inputs and returns the FULL output.
"""

import os
from contextlib import ExitStack

import numpy as np

import concourse.bass as bass
import concourse.bacc as bacc
import concourse.mybir as mybir
import concourse.tile as tile

F32 = mybir.dt.float32
AF = mybir.ActivationFunctionType
OP = mybir.AluOpType

B = 8192
OBS = 64
ACTD = 16
H = 128
COMP = 85
KS = 32
NCORES = 8
S = B // NCORES        # 1024 samples per core
NB = S // 128          # 8 sample blocks of 128
CH = 512               # matmul moving-operand chunk (one PSUM bank of f32)
EPS = 1e-4

# name -> (shape, bf16?) of every replicated weight, packed host-side into
# two [128, N] arrays (one f32, one bf16) so the kernel needs just 2 DMAs
_WSLOTS = {
    "ow1T": ([OBS, H], 0), "ob1": ([H, 1], 0), "ow2T": ([H, H], 0),
    "ob2": ([H, 1], 0), "aw1T": ([ACTD, H], 0), "ab1": ([H, 1], 0),
    "aw2T": ([H, H], 0), "ab2": ([H, 1], 0),
    "kw1Ta": ([H, KS], 0), "kw1Tb": ([H, KS], 0), "kb1": ([KS, 1], 0),
    "kw2T": ([KS, KS // 2], 0), "kb2": ([KS // 2, 1], 0),
    "kw3T": ([KS // 2, 1], 0), "kb3": ([1, 1], 0),
    "mw1Ta": ([H, COMP], 0), "mw1Tb": ([H, COMP], 0), "mb1": ([COMP, 1], 0),
    "mw2T": ([COMP, COMP], 0), "mb2": ([COMP, 1], 0),
    "mw3T": ([COMP, COMP], 0), "mb3": ([COMP, 1], 0),
    "ones": ([COMP, 1], 0), "iden": ([H, H], 0),
    "mall": ([COMP, COMP * COMP], 1), "mw3Tb": ([COMP, COMP], 1),
    "idenb": ([COMP, COMP], 1), "onesb": ([COMP, 1], 1),
}
_OFFS = {}
_NCOLS = [0, 0]
for _n, (_shp, _b) in _WSLOTS.items():
    _OFFS[_n] = _NCOLS[_b]
    _NCOLS[_b] += _shp[1]

def host_prep(inputs):
    """Host-side weight preprocessing + packing (pure numpy, all tiny)."""
    import ml_dtypes
    f = lambda a: np.ascontiguousarray(np.asarray(a, dtype=np.float32))
    W1, W2, W3 = f(inputs["mw1"]), f(inputs["mw2"]), f(inputs["mw3"])
    G = (W1 @ W1.T).astype(np.float64)
    lam, U = np.linalg.eigh(G)
    L = (U * np.sqrt(np.clip(lam, 0.0, None))).astype(np.float32)  # G = L L^T
    # mall[l, c*85+j] = W2[j, l] * L[l, c]   (stage-1 stationary lhsT per c)
    mall = (W2.T[:, None, :] * L[:, :, None]).reshape(COMP, COMP * COMP)
    vals = {
        "ow1T": f(inputs["ow1"]).T, "ob1": f(inputs["ob1"]).reshape(H, 1),
        "ow2T": f(inputs["ow2"]).T, "ob2": f(inputs["ob2"]).reshape(H, 1),
        "aw1T": f(inputs["aw1"]).T, "ab1": f(inputs["ab1"]).reshape(H, 1),
        "aw2T": f(inputs["aw2"]).T, "ab2": f(inputs["ab2"]).reshape(H, 1),
        "kw1Ta": f(inputs["kw1"]).T[:H], "kw1Tb": f(inputs["kw1"]).T[H:],
        "kb1": f(inputs["kb1"]).reshape(KS, 1),
        "kw2T": f(inputs["kw2"]).T, "kb2": f(inputs["kb2"]).reshape(KS // 2, 1),
        "kw3T": f(inputs["kw3"]).T, "kb3": f(inputs["kb3"]).reshape(1, 1),
        "mw1Ta": W1.T[:H], "mw1Tb": W1.T[H:],
        "mb1": f(inputs["mb1"]).reshape(COMP, 1),
        "mw2T": W2.T, "mb2": f(inputs["mb2"]).reshape(COMP, 1),
        "mw3T": W3.T, "mb3": f(inputs["mb3"]).reshape(COMP, 1),
        "ones": np.ones((COMP, 1), np.float32),
        "iden": np.eye(H, dtype=np.float32),
        "mall": mall, "mw3Tb": W3.T,
        "idenb": np.eye(COMP, dtype=np.float32),
        "onesb": np.ones((COMP, 1), np.float32),
    }
    packs = [np.zeros((128, _NCOLS[0]), np.float32),
             np.zeros((128, _NCOLS[1]), ml_dtypes.bfloat16)]
    for n, (shp, b) in _WSLOTS.items():
        o = _OFFS[n]
        packs[b][:shp[0], o:o + shp[1]] = vals[n]
    return {"wpack32": packs[0], "wpack16": packs[1]}


def build_nc(reps=1):
    nc = bacc.Bacc()

    obs_d = nc.declare_dram_parameter("obs", [S, OBS], F32, isOutput=False)
    act_d = nc.declare_dram_parameter("action", [S, ACTD], F32, isOutput=False)
    BF16 = mybir.dt.bfloat16
    wp32_d = nc.declare_dram_parameter("wpack32", [128, _NCOLS[0]], F32,
                                       isOutput=False)
    wp16_d = nc.declare_dram_parameter("wpack16", [128, _NCOLS[1]], BF16,
                                       isOutput=False)
    tick_d = nc.declare_dram_parameter("tick", [1, 1], F32, isOutput=False)
    out_d = nc.declare_dram_parameter("out", [S, COMP], F32, isOutput=True)

    with tile.TileContext(nc) as tc, ExitStack() as ctx:
        wp = ctx.enter_context(tc.tile_pool(name="weights", bufs=1))
        ap = ctx.enter_context(tc.tile_pool(name="acts", bufs=1))
        zp = ctx.enter_context(tc.tile_pool(name="zbuf", bufs=4))
        sqp = ctx.enter_context(tc.tile_pool(name="sqbuf", bufs=4))
        outp = ctx.enter_context(tc.tile_pool(name="outbuf", bufs=3))
        smp = ctx.enter_context(tc.tile_pool(name="small", bufs=16))
        psA = ctx.enter_context(tc.tile_pool(name="psA", bufs=3, space="PSUM"))
        psC = ctx.enter_context(tc.tile_pool(name="psC", bufs=1, space="PSUM"))

        # ---- load weights (2 packed DMAs), expose per-weight slice views ----
        wp32 = wp.tile([128, _NCOLS[0]], F32, tag="wp32", name="wp32")
        wp16 = wp.tile([128, _NCOLS[1]], BF16, tag="wp16", name="wp16")
        nc.sync.dma_start(wp32[:], wp32_d[:])
        w = {}
        for name, (shp, b) in _WSLOTS.items():
            o = _OFFS[name]
            w[name] = (wp16 if b else wp32)[0:shp[0], o:o + shp[1]]

        tick_sb = wp.tile([1, 1], F32, tag="tick_sb", name="tick_sb")
        nc.sync.dma_start(tick_sb[:], tick_d[:])

        # ---- load + transpose obs/action into [feat, S] layout ----
        for _rep in range(reps):
            obs_sb = ap.tile([128, NB, OBS], F32, tag="obs_sb")
            act_sb = ap.tile([128, NB, ACTD], F32, tag="act_sb")
            for nb in range(NB):
                nc.sync.dma_start(obs_sb[:, nb, :], obs_d[nb * 128:(nb + 1) * 128, :])
                nc.sync.dma_start(act_sb[:, nb, :], act_d[nb * 128:(nb + 1) * 128, :])
            # collapse the many DMA-queue semaphores into one barrier so no
            # matmul needs more than one sync wait (walrus S3_LW limit)
            tc.strict_bb_all_engine_barrier()
            # the big bf16 pack (stage-1 matrices) is only needed at J-loop
            # start; issued after the barrier so the forward overlaps it
            nc.sync.dma_start(wp16[:], wp16_d[:])

            obst = ap.tile([OBS, S], F32, tag="obst")
            actt = ap.tile([ACTD, S], F32, tag="actt")
            for nb in range(NB):
                pt = psA.tile([OBS, 128], F32, tag="a")
                nc.tensor.transpose(pt[:], obs_sb[:, nb, :], w["iden"][:])
                nc.vector.tensor_copy(obst[:, nb * 128:(nb + 1) * 128], pt[:])
                pt2 = psA.tile([ACTD, 128], F32, tag="a")
                nc.tensor.transpose(pt2[:], act_sb[:, nb, :], w["iden"][:])
                nc.vector.tensor_copy(actt[:, nb * 128:(nb + 1) * 128], pt2[:])

            # ---- forward layers ([feat, S], chunked matmuls + fused ACT) ----
            def layer(dst, dst_sl, terms, bias, func, p):
                # dst[dst_sl] = func(sum_i lhsT_i.T @ rhs_i + bias), chunked over S
                m = dst.shape[-1] if dst_sl is None else None
                for ch in range(S // CH):
                    sl = slice(ch * CH, (ch + 1) * CH)
                    pt = p.tile([terms[0][0].shape[-1], CH], F32, tag="a", name="pt")
                    n = len(terms)
                    for i, (lhsT, rhs) in enumerate(terms):
                        nc.tensor.matmul(pt[:], lhsT[:], rhs[:, sl],
                                         start=(i == 0), stop=(i == n - 1))
                    dsl = dst[:, sl] if dst_sl is None else dst[dst_sl, sl]
                    if func == AF.Relu:
                        nc.vector.tensor_scalar(out=dsl, in0=pt[:], scalar1=bias[:],
                                                scalar2=0.0, op0=OP.add, op1=OP.max)
                    else:
                        nc.scalar.activation(dsl, pt[:], func, bias=bias[:])

            oh1 = ap.tile([H, S], F32, tag="oh1")
            layer(oh1, None, [(w["ow1T"], obst)], w["ob1"], AF.Relu, psA)
            of = ap.tile([H, S], F32, tag="of")
            layer(of, None, [(w["ow2T"], oh1)], w["ob2"], AF.Relu, psA)
            ah1 = ap.tile([H, S], F32, tag="ah1")
            layer(ah1, None, [(w["aw1T"], actt)], w["ab1"], AF.Relu, psA)
            af = ap.tile([H, S], F32, tag="af")
            layer(af, None, [(w["aw2T"], ah1)], w["ab2"], AF.Relu, psA)

            k1 = ap.tile([KS, S], F32, tag="k1")
            layer(k1, None, [(w["kw1Ta"], of), (w["kw1Tb"], af)], w["kb1"], AF.Tanh, psA)
            k2 = ap.tile([KS // 2, S], F32, tag="k2")
            layer(k2, None, [(w["kw2T"], k1)], w["kb2"], AF.Tanh, psA)

            # k_out = softplus(kw3 @ k2 + kb3) = ln(1 + exp(.)) via Exp then Ln(x+1)
            kexp = ap.tile([1, S], F32, tag="kexp")
            layer(kexp, None, [(w["kw3T"], k2)], w["kb3"], AF.Exp, psA)
            kout = ap.tile([1, S], F32, tag="kout")
            nc.scalar.activation(kout[:], kexp[:], AF.Ln, bias=1.0)

            h1 = ap.tile([COMP, S], F32, tag="h1")
            layer(h1, None, [(w["mw1Ta"], of), (w["mw1Tb"], af)], w["mb1"], AF.Relu, psA)
            d1 = ap.tile([COMP, S], BF16, tag="d1")
            nc.vector.tensor_scalar(out=d1[:], in0=h1[:], scalar1=0.0, scalar2=None,
                                    op0=OP.is_gt)
            h2 = ap.tile([COMP, S], F32, tag="h2")
            layer(h2, None, [(w["mw2T"], h1)], w["mb2"], AF.Relu, psA)
            d2 = ap.tile([COMP, S], F32, tag="d2")
            nc.vector.tensor_scalar(out=d2[:], in0=h2[:], scalar1=0.0, scalar2=None,
                                    op0=OP.is_gt)
            fout = ap.tile([COMP, S], F32, tag="fout")
            layer(fout, None, [(w["mw3T"], h2)], w["mb3"], AF.Relu, psA)
            d3 = ap.tile([COMP, S], F32, tag="d3")
            nc.vector.tensor_scalar(out=d3[:], in0=fout[:], scalar1=0.0,
                                    scalar2=None, op0=OP.is_gt)

            # ---- Jacobian-norm loop over the 85 columns of L ----
            # bf16 identity for the accumulate-matmul (fp32 matmuls lower to
            # HI/LO pairs that break inside an interleaved accumulation group)
            idenb = w["idenb"]
            accp = psC.tile([COMP, S], F32, tag="c")   # persistent PSUM accumulator
            ACCs = ap.tile([COMP, S], F32, tag="ACCs")  # SBUF spill of acc groups
            GRP = 28   # accumulation-group length (bounded for HW robustness)
            acc_n = [0]

            def acc_mm(sq):
                n = acc_n[0]
                for ch in range(S // CH):
                    sl = slice(ch * CH, (ch + 1) * CH)
                    nc.tensor.matmul(accp[:, sl], idenb[:], sq[:, sl],
                                     start=(n % GRP == 0),
                                     stop=(n % GRP == GRP - 1 or n == COMP - 1),
                                     skip_group_check=True)
                acc_n[0] = n + 1
                if n % GRP == GRP - 1 or n == COMP - 1:
                    if n < GRP:
                        nc.vector.tensor_copy(ACCs[:], accp[:])
                    else:
                        nc.vector.tensor_tensor(ACCs[:], accp[:], ACCs[:], OP.add)

            tc.strict_bb_all_engine_barrier()
            # software pipeline: py prefetched one c ahead of the DVE mask,
            # squares accumulated two c behind, so PE never heads-of-line
            # blocks the mask -> pr -> py -> mask cycle
            pys = {}

            def emit_py(c):
                t = psA.tile([COMP, S], F32, tag="a", name="py")
                for ch in range(S // CH):
                    sl = slice(ch * CH, (ch + 1) * CH)
                    nc.tensor.matmul(t[:, sl], w["mall"][:, c * COMP:(c + 1) * COMP],
                                     d1[:, sl], start=True, stop=True)
                pys[c] = t

            emit_py(0)
            pend = []
            for c in range(COMP):
                z = zp.tile([COMP, S], BF16, tag="z")
                nc.vector.tensor_tensor(z[:], pys.pop(c)[:], d2[:], OP.mult)
                if c + 1 < COMP:
                    emit_py(c + 1)
                if len(pend) == 2:
                    acc_mm(pend.pop(0))
                pr = psA.tile([COMP, S], F32, tag="a", name="pr")
                for ch in range(S // CH):
                    sl = slice(ch * CH, (ch + 1) * CH)
                    nc.tensor.matmul(pr[:, sl], w["mw3Tb"][:], z[:, sl],
                                     start=True, stop=True)
                sq = sqp.tile([COMP, S], BF16, tag="sq")
                nc.scalar.square(sq[:], pr[:])
                pend.append(sq)
            acc_mm(pend.pop(0))
            acc_mm(pend.pop(0))

            # ---- finale: jn2 = ones^T (d3 * acc); out = tanh(kout*fout/(sqrt+eps)) ----
            am = zp.tile([COMP, S], BF16, tag="am")
            nc.vector.tensor_tensor(am[:], ACCs[:], d3[:], OP.mult)
            pj = psA.tile([1, S], F32, tag="a", name="pj")
            pj_lhs = w["onesb"]
            for ch in range(S // CH):
                sl = slice(ch * CH, (ch + 1) * CH)
                nc.tensor.matmul(pj[:, sl], pj_lhs[:], am[:, sl],
                                 start=True, stop=True)
            jn2 = ap.tile([1, S], F32, tag="jn2")
            nc.scalar.copy(jn2[:], pj[:])

            tc.strict_bb_all_engine_barrier()

            # batch the per-sample scale: transpose jn2/kout for all blocks
            # into one [128, 2*NB] tile, then one sqrt + vector recip pass
            pjk = psA.tile([128, 2 * NB], F32, tag="a", name="pjk")
            for nb in range(NB):
                sl = slice(nb * 128, (nb + 1) * 128)
                nc.tensor.transpose(pjk[:, nb:nb + 1], jn2[:, sl], w["iden"][:1, :1])
                nc.tensor.transpose(pjk[:, NB + nb:NB + nb + 1], kout[:, sl],
                                    w["iden"][:1, :1])
            den = smp.tile([128, NB], F32, tag="den")
            nc.scalar.activation(den[:], pjk[:, 0:NB], AF.Sqrt)
            rec = smp.tile([128, NB], F32, tag="rec")
            nc.vector.tensor_scalar_add(rec[:], den[:], EPS)
            nc.vector.reciprocal(rec[:], rec[:])
            scl = smp.tile([128, NB], F32, tag="scl")
            nc.vector.tensor_tensor(scl[:], rec[:], pjk[:, NB:2 * NB], OP.mult)
            for nb in range(NB):
                sl = slice(nb * 128, (nb + 1) * 128)
                pt = psA.tile([128, COMP], F32, tag="a", name="ptf")
                nc.tensor.transpose(pt[:], fout[:, sl], w["iden"][:COMP, :COMP])
                ot = outp.tile([128, COMP], F32, tag="ot")
                nc.scalar.activation(ot[:], pt[:], AF.Tanh, scale=scl[:, nb:nb + 1])
                nc.sync.dma_start(out_d[sl, :], ot[:])

    return nc


_NC = None


def _get_nc():
    global _NC
    if _NC is None:
        _NC = build_nc()
        _NC.finalize()
    return _NC


def make_in_maps(inputs):
    w = host_prep(inputs)
    obs = np.ascontiguousarray(np.asarray(inputs["obs"], np.float32))
    act = np.ascontiguousarray(np.asarray(inputs["action"], np.float32))
    in_maps = []
    for i in range(NCORES):
        m = dict(w)
        m["obs"] = np.ascontiguousarray(obs[i * S:(i + 1) * S])
        m["action"] = np.ascontiguousarray(act[i * S:(i + 1) * S])
        m["tick"] = np.zeros((1, 1), np.float32)
        in_maps.append(m)
    return in_maps


def kernel(**inputs):
    from concourse.bass_utils import run_bass_kernel_spmd

    nc = _get_nc()
    in_maps = make_in_maps(inputs)
    res = run_bass_kernel_spmd(nc, in_maps, core_ids=list(range(NCORES)))
    return np.concatenate([r["out"] for r in res.results], axis=0)



# revision 2
# speedup vs baseline: 2.5512x; 1.0379x over previous
"""Trainium2 Bass kernel for the lipsnet CustomModel problem.

Math: the reference computes, per sample,
    jac_norm = ||D3 W3 D2 W2 D1 W1||_F      (Di = diag(relu'(pi)))
    out = tanh(k_out * f_out / (jac_norm + 1e-4))
Key identity:  with G = W1 W1^T = L L^T (host eigen factorization),
    ||D3 W3 D2 W2 D1 W1||_F^2 = sum_c || D3 W3 D2 (M_c @ d1) ||^2
where M_c[j,l] = W2[j,l] * L[l,c] are 85 host-precomputed stationary
matrices and d1/d2/d3 are the per-sample binary relu masks.  Every
per-sample 85x85x85 contraction becomes a stationary-weight matmul with
the mask tensor as the moving operand.

This version vs the original baseline:
  * all forward matmuls in bf16 (1 cycle/row instead of 4 for fp32)
  * stage-1 of the J-loop runs in fp8e4m3 with DoubleRow perf mode
    (0.5 cycles/row): the contraction index j is folded to [43, 2] on
    the partition dim, which only d1/h1 production has to know about
  * constant weights are DMA'd once, outside the rep loop
  * d-mask production and the final d3-mask multiply run on the (idle)
    GpSimd engine to keep DVE free for the per-c mask multiplies
  * softplus(k) is evaluated after the [1,S] -> [128,NB] transpose
    so the ACT work is on 8-wide tiles, not 1024-wide

Sharding: pure data parallel over the batch dim, 8 NeuronCores, weights
replicated.  kernel() takes FULL inputs and returns the FULL output.
"""

from contextlib import ExitStack

import numpy as np

import concourse.bass as bass
import concourse.bacc as bacc
import concourse.mybir as mybir
import concourse.tile as tile

F32 = mybir.dt.float32
BF16 = mybir.dt.bfloat16
F8 = mybir.dt.float8e4
AF = mybir.ActivationFunctionType
OP = mybir.AluOpType
DR = mybir.MatmulPerfMode.DoubleRow

B = 8192
OBS = 64
ACTD = 16
H = 128
COMP = 85
KS = 32
NCORES = 8
S = B // NCORES        # 1024 samples per core
NB = S // 128          # 8 sample blocks of 128
CH = 512               # matmul moving-operand chunk (one PSUM bank of f32)
JH = 43                # folded half of the j index (2*43 = 86 >= 85)
EPS = 1e-4
S1 = 64.0              # fp8 scale on the stage-1 stationary (power of 2)
GRP = 28               # PSUM accumulation-group length for the c-accumulate

# ---- packed replicated weights: (shape, pack) with pack 0=f32 1=bf16 ----
_WSLOTS = {
    # f32: biases (per-partition scalars), transpose identity
    "ob1": ([H, 1], 0), "ob2": ([H, 1], 0),
    "ab1": ([H, 1], 0), "ab2": ([H, 1], 0),
    "kb1": ([KS, 1], 0), "kb2": ([KS // 2, 1], 0), "kb3": ([1, 1], 0),
    "mb1f": ([JH, 2], 0), "mb2": ([COMP, 1], 0), "mb3": ([COMP, 1], 0),
    "iden": ([H, H], 0),
    # bf16: all matmul stationaries
    "ow1T": ([OBS, H], 1), "ow2T": ([H, H], 1),
    "aw1T": ([ACTD, H], 1), "aw2T": ([H, H], 1),
    "kw1Ta": ([H, KS], 1), "kw1Tb": ([H, KS], 1),
    "kw2T": ([KS, KS // 2], 1), "kw3T": ([KS // 2, 1], 1),
    "mw1Taf": ([H, 2 * JH], 1), "mw1Tbf": ([H, 2 * JH], 1),
    "mw2f": ([JH, 2 * COMP], 1),
    "mw3T": ([COMP, COMP], 1),
    "idenb": ([COMP, COMP], 1), "onesb": ([COMP, 1], 1),
    "idenc": ([H, H], 1),
}
_OFFS = {}
_NCOLS = [0, 0]
for _n, (_shp, _b) in _WSLOTS.items():
    _OFFS[_n] = _NCOLS[_b]
    _NCOLS[_b] += _shp[1]


def host_prep(inputs):
    """Host-side weight preprocessing + packing (pure numpy, all tiny)."""
    f = lambda a: np.ascontiguousarray(np.asarray(a, dtype=np.float32))
    W1, W2, W3 = f(inputs["mw1"]), f(inputs["mw2"]), f(inputs["mw3"])
    G = (W1 @ W1.T).astype(np.float64)
    lam, U = np.linalg.eigh(G)
    L = (U * np.sqrt(np.clip(lam, 0.0, None))).astype(np.float32)  # G = L L^T

    # folded j layout: j = t*JH + p, p in [0,JH), t in {0,1}; j==85 is a pad
    def fold_cols(m):  # [rows, 85] -> [rows, 2, JH] zero-padded
        out = np.zeros((m.shape[0], 2, JH), np.float32)
        out[:, 0, :] = m[:, :JH]
        out[:, 1, : COMP - JH] = m[:, JH:COMP]
        return out

    def fold_rows(m):  # [85, cols] -> [2, JH, cols] zero-padded
        out = np.zeros((2, JH, m.shape[1]), np.float32)
        out[0, :, :] = m[:JH]
        out[1, : COMP - JH, :] = m[JH:COMP]
        return out

    W1T = W1.T  # [256, 85]
    mw1Taf = fold_cols(W1T[:H]).reshape(H, 2 * JH)
    mw1Tbf = fold_cols(W1T[H:]).reshape(H, 2 * JH)
    mb1f = fold_cols(f(inputs["mb1"]).reshape(1, COMP))[0].T  # [JH, 2]
    mw2f = fold_rows(W2.T).transpose(1, 0, 2).reshape(JH, 2 * COMP)

    # mall8[p, t, c, m] = S1 * W2[m, j] * L[j, c], j = t*JH + p
    mall = np.zeros((JH, 2, COMP, COMP), np.float32)
    for t in range(2):
        n = JH if t == 0 else COMP - JH
        j = np.arange(n) + t * JH
        # [n, c, m] = L[j, c] * W2[m, j]
        mall[:n, t] = L[j][:, :, None] * W2.T[j][:, None, :]
    f8np = mybir.dt.np(F8)
    mall8 = (mall.reshape(JH, 2 * COMP * COMP) * S1).astype(f8np)

    bf = mybir.dt.np(BF16)
    vals32 = {
        "ob1": f(inputs["ob1"]).reshape(H, 1), "ob2": f(inputs["ob2"]).reshape(H, 1),
        "ab1": f(inputs["ab1"]).reshape(H, 1), "ab2": f(inputs["ab2"]).reshape(H, 1),
        "kb1": f(inputs["kb1"]).reshape(KS, 1),
        "kb2": f(inputs["kb2"]).reshape(KS // 2, 1),
        "kb3": f(inputs["kb3"]).reshape(1, 1),
        "mb1f": mb1f, "mb2": f(inputs["mb2"]).reshape(COMP, 1),
        "mb3": f(inputs["mb3"]).reshape(COMP, 1),
        "iden": np.eye(H, dtype=np.float32),
    }
    vals16 = {
        "ow1T": f(inputs["ow1"]).T, "ow2T": f(inputs["ow2"]).T,
        "aw1T": f(inputs["aw1"]).T, "aw2T": f(inputs["aw2"]).T,
        "kw1Ta": f(inputs["kw1"]).T[:H], "kw1Tb": f(inputs["kw1"]).T[H:],
        "kw2T": f(inputs["kw2"]).T, "kw3T": f(inputs["kw3"]).T,
        "mw1Taf": mw1Taf, "mw1Tbf": mw1Tbf, "mw2f": mw2f,
        "mw3T": W3.T,
        "idenb": np.eye(COMP, dtype=np.float32),
        "onesb": np.ones((COMP, 1), np.float32),
        "idenc": np.eye(H, dtype=np.float32),
    }
    packs = [np.zeros((128, _NCOLS[0]), np.float32),
             np.zeros((128, _NCOLS[1]), bf)]
    for n, (shp, b) in _WSLOTS.items():
        o = _OFFS[n]
        packs[b][: shp[0], o : o + shp[1]] = (vals32 if b == 0 else vals16)[n]
    return {"wpack32": packs[0], "wpack16": packs[1], "wpack8": mall8}


def build_nc(reps=1):
    nc = bacc.Bacc()

    obs_d = nc.declare_dram_parameter("obs", [S, OBS], F32, isOutput=False)
    act_d = nc.declare_dram_parameter("action", [S, ACTD], F32, isOutput=False)
    wp32_d = nc.declare_dram_parameter("wpack32", [128, _NCOLS[0]], F32,
                                       isOutput=False)
    wp16_d = nc.declare_dram_parameter("wpack16", [128, _NCOLS[1]], BF16,
                                       isOutput=False)
    wp8_d = nc.declare_dram_parameter("wpack8", [JH, 2 * COMP * COMP], F8,
                                      isOutput=False)
    out_d = nc.declare_dram_parameter("out", [S, COMP], F32, isOutput=True)

    with tile.TileContext(nc) as tc, ExitStack() as ctx:
        wp = ctx.enter_context(tc.tile_pool(name="weights", bufs=1))
        inp = ctx.enter_context(tc.tile_pool(name="inbuf", bufs=1))
        ap = ctx.enter_context(tc.tile_pool(name="acts", bufs=1))
        zp = ctx.enter_context(tc.tile_pool(name="zbuf", bufs=4))
        sqp = ctx.enter_context(tc.tile_pool(name="sqbuf", bufs=4))
        outp = ctx.enter_context(tc.tile_pool(name="outbuf", bufs=3))
        smp = ctx.enter_context(tc.tile_pool(name="small", bufs=16))
        psA = ctx.enter_context(tc.tile_pool(name="psA", bufs=3, space="PSUM"))
        psC = ctx.enter_context(tc.tile_pool(name="psC", bufs=1, space="PSUM"))

        # ---- load weights once (3 packed DMAs), expose slice views ----
        wp32 = wp.tile([128, _NCOLS[0]], F32, tag="wp32", name="wp32")
        wp16 = wp.tile([128, _NCOLS[1]], BF16, tag="wp16", name="wp16")
        mall8 = wp.tile([JH, 2, COMP, COMP], F8, tag="mall8", name="mall8")
        nc.sync.dma_start(wp32[:], wp32_d[:])
        nc.sync.dma_start(wp16[:], wp16_d[:])
        nc.sync.dma_start(
            mall8[:], wp8_d[:].rearrange("p (t c m) -> p t c m", t=2, c=COMP))
        w = {}
        for name, (shp, b) in _WSLOTS.items():
            o = _OFFS[name]
            w[name] = (wp16 if b else wp32)[0 : shp[0], o : o + shp[1]]

        for _rep in range(reps):
            # ---- load obs/action, transpose into [feat, S] bf16 ----
            obs_sb = inp.tile([128, NB, OBS], F32, tag="obs_sb")
            act_sb = inp.tile([128, NB, ACTD], F32, tag="act_sb")
            for nb in range(NB):
                nc.sync.dma_start(obs_sb[:, nb, :], obs_d[nb * 128:(nb + 1) * 128, :])
                nc.sync.dma_start(act_sb[:, nb, :], act_d[nb * 128:(nb + 1) * 128, :])
            tc.strict_bb_all_engine_barrier()

            obst = ap.tile([OBS, S], BF16, tag="obst")
            actt = ap.tile([ACTD, S], BF16, tag="actt")
            pto = psA.tile([OBS, 2, NB // 2, 128], F32, tag="a", name="pto")
            pta = psA.tile([ACTD, NB, 128], F32, tag="a", name="pta")
            for nb in range(NB):
                nc.tensor.transpose(pto[:, nb // (NB // 2), nb % (NB // 2), :],
                                    obs_sb[:, nb, :], w["iden"][:])
                nc.tensor.transpose(pta[:, nb, :], act_sb[:, nb, :], w["iden"][:])
            nc.scalar.copy(obst[:].rearrange("f (t h s) -> f t h s", t=2, h=NB // 2),
                           pto[:])
            nc.scalar.copy(actt[:].rearrange("f (h s) -> f h s", h=NB), pta[:])

            # ---- forward layers ([feat, S], chunked bf16 matmuls) ----
            def layer(dst, terms, bias, func, eng="v"):
                # dst = func(sum_i lhsT_i.T @ rhs_i + bias), chunked over S
                for ch in range(S // CH):
                    sl = slice(ch * CH, (ch + 1) * CH)
                    pt = psA.tile([terms[0][0].shape[-1], CH], F32, tag="a",
                                  name="pt")
                    n = len(terms)
                    for i, (lhsT, rhs) in enumerate(terms):
                        nc.tensor.matmul(pt[:], lhsT[:], rhs[:, sl],
                                         start=(i == 0), stop=(i == n - 1))
                    if func == AF.Relu and eng == "v":
                        nc.vector.tensor_scalar(out=dst[:, sl], in0=pt[:],
                                                scalar1=bias[:], scalar2=0.0,
                                                op0=OP.add, op1=OP.max)
                    else:
                        nc.scalar.activation(dst[:, sl], pt[:], func, bias=bias[:])

            oh1 = ap.tile([H, S], BF16, tag="oh1")
            layer(oh1, [(w["ow1T"], obst)], w["ob1"], AF.Relu)
            of = ap.tile([H, S], BF16, tag="of")
            layer(of, [(w["ow2T"], oh1)], w["ob2"], AF.Relu)
            ah1 = ap.tile([H, S], BF16, tag="ah1")
            layer(ah1, [(w["aw1T"], actt)], w["ab1"], AF.Relu)
            af = ap.tile([H, S], BF16, tag="af")
            layer(af, [(w["aw2T"], ah1)], w["ab2"], AF.Relu)

            k1 = ap.tile([KS, S], BF16, tag="k1")
            layer(k1, [(w["kw1Ta"], of), (w["kw1Tb"], af)], w["kb1"], AF.Tanh,
                  eng="s")
            k2 = ap.tile([KS // 2, S], BF16, tag="k2")
            layer(k2, [(w["kw2T"], k1)], w["kb2"], AF.Tanh, eng="s")
            # kpre = kw3 @ k2 + kb3, [1, S]; softplus happens post-transpose
            kpre = ap.tile([1, S], F32, tag="kpre")
            layer(kpre, [(w["kw3T"], k2)], w["kb3"], AF.Identity, eng="s")

            # h1 in folded-j layout [JH, 2, S]; d1 mask in fp8 for DoubleRow
            h1f = ap.tile([JH, 2, S], BF16, tag="h1f")
            for ch in range(S // CH):
                sl = slice(ch * CH, (ch + 1) * CH)
                pt1 = psA.tile([JH, 2, CH], F32, tag="a", name="pt1")
                for t in range(2):
                    tsl = slice(t * JH, (t + 1) * JH)
                    nc.tensor.matmul(pt1[:, t, :], w["mw1Taf"][:, tsl],
                                     of[:, sl], start=True, stop=False)
                    nc.tensor.matmul(pt1[:, t, :], w["mw1Tbf"][:, tsl],
                                     af[:, sl], start=False, stop=True)
                for t in range(2):
                    nc.vector.tensor_scalar(out=h1f[:, t, sl], in0=pt1[:, t, :],
                                            scalar1=w["mb1f"][:, t : t + 1],
                                            scalar2=0.0, op0=OP.add, op1=OP.max)
            d1q = ap.tile([JH, 2, S], F8, tag="d1q")
            nc.gpsimd.tensor_scalar(out=d1q[:, 0, :], in0=h1f[:, 0, :],
                                    scalar1=0.0, scalar2=None, op0=OP.is_gt)
            nc.vector.tensor_scalar(out=d1q[:, 1, :], in0=h1f[:, 1, :],
                                    scalar1=0.0, scalar2=None, op0=OP.is_gt)

            h2 = ap.tile([COMP, S], BF16, tag="h2")
            layer(h2, [(w["mw2f"][:, :COMP], h1f[:, 0, :]),
                       (w["mw2f"][:, COMP:], h1f[:, 1, :])], w["mb2"], AF.Relu)
            d2 = ap.tile([COMP, S], BF16, tag="d2")
            nc.gpsimd.tensor_scalar(out=d2[:], in0=h2[:], scalar1=0.0,
                                    scalar2=None, op0=OP.is_gt)
            fout = ap.tile([COMP, S], BF16, tag="fout")
            layer(fout, [(w["mw3T"], h2)], w["mb3"], AF.Relu, eng="s")
            d3 = ap.tile([COMP, S], BF16, tag="d3")
            nc.gpsimd.tensor_scalar(out=d3[:], in0=fout[:], scalar1=0.0,
                                    scalar2=None, op0=OP.is_gt)

            # ---- Jacobian-norm loop over the 85 columns of L ----
            accp = psC.tile([COMP, S], F32, tag="c")   # persistent PSUM acc
            ACCs = ap.tile([COMP, S], F32, tag="ACCs")  # SBUF spill of groups
            acc_n = [0]

            def acc_mm(sq):
                n = acc_n[0]
                for ch in range(S // CH):
                    sl = slice(ch * CH, (ch + 1) * CH)
                    nc.tensor.matmul(accp[:, sl], w["idenb"][:], sq[:, sl],
                                     start=(n % GRP == 0),
                                     stop=(n % GRP == GRP - 1 or n == COMP - 1),
                                     skip_group_check=True)
                acc_n[0] = n + 1
                if n % GRP == GRP - 1 or n == COMP - 1:
                    if n < GRP:
                        nc.vector.tensor_copy(ACCs[:], accp[:])
                    else:
                        nc.vector.tensor_tensor(ACCs[:], accp[:], ACCs[:], OP.add)

            tc.strict_bb_all_engine_barrier()
            # software pipeline: py prefetched one c ahead of the DVE mask,
            # squares accumulated two c behind
            pys = {}

            def emit_py(c):
                t = psA.tile([COMP, S], F32, tag="a", name="py")
                for ch in range(S // CH):
                    sl = slice(ch * CH, (ch + 1) * CH)
                    nc.tensor.matmul(t[:, sl], mall8[:, :, c, :],
                                     d1q[:, :, sl], start=True, stop=True,
                                     perf_mode=DR)
                pys[c] = t

            emit_py(0)
            pend = []
            for c in range(COMP):
                z = zp.tile([COMP, S], BF16, tag="z")
                nc.vector.tensor_tensor(z[:], pys.pop(c)[:], d2[:], OP.mult)
                if c + 1 < COMP:
                    emit_py(c + 1)
                if len(pend) == 2:
                    acc_mm(pend.pop(0))
                pr = psA.tile([COMP, S], F32, tag="a", name="pr")
                for ch in range(S // CH):
                    sl = slice(ch * CH, (ch + 1) * CH)
                    nc.tensor.matmul(pr[:, sl], w["mw3T"][:], z[:, sl],
                                     start=True, stop=True)
                sq = sqp.tile([COMP, S], BF16, tag="sq")
                nc.scalar.square(sq[:], pr[:])
                pend.append(sq)
            acc_mm(pend.pop(0))
            acc_mm(pend.pop(0))

            # ---- finale: jn2 = ones^T (d3 * acc); out = tanh(kout*fout/den) ----
            am = zp.tile([COMP, S], BF16, tag="am")
            nc.gpsimd.tensor_tensor(am[:], ACCs[:], d3[:], OP.mult)
            pj = psA.tile([1, S], F32, tag="a", name="pj")
            for ch in range(S // CH):
                sl = slice(ch * CH, (ch + 1) * CH)
                nc.tensor.matmul(pj[:, sl], w["onesb"][:], am[:, sl],
                                 start=True, stop=True)
            jn2 = ap.tile([1, S], F32, tag="jn2")
            nc.scalar.copy(jn2[:], pj[:])

            tc.strict_bb_all_engine_barrier()

            # batch the per-sample scale: transpose jn2/kpre for all blocks
            # into one [128, 2*NB] tile, then narrow ACT/DVE passes
            pjk = psA.tile([128, 2 * NB], F32, tag="a", name="pjk")
            for nb in range(NB):
                sl = slice(nb * 128, (nb + 1) * 128)
                nc.tensor.transpose(pjk[:, nb : nb + 1], jn2[:, sl],
                                    w["iden"][:1, :1])
                nc.tensor.transpose(pjk[:, NB + nb : NB + nb + 1], kpre[:, sl],
                                    w["iden"][:1, :1])
            # den = sqrt(jn2)/S1 + EPS ; kout = ln(1+exp(kpre)); scl = kout/den
            den = smp.tile([128, NB], F32, tag="den")
            nc.scalar.activation(den[:], pjk[:, 0:NB], AF.Sqrt,
                                 scale=1.0 / (S1 * S1))
            kex = smp.tile([128, NB], F32, tag="kex")
            nc.scalar.activation(kex[:], pjk[:, NB : 2 * NB], AF.Exp)
            kout = smp.tile([128, NB], F32, tag="kout")
            nc.scalar.activation(kout[:], kex[:], AF.Ln, bias=1.0)
            rec = smp.tile([128, NB], F32, tag="rec")
            nc.vector.tensor_scalar_add(rec[:], den[:], EPS)
            nc.vector.reciprocal(rec[:], rec[:])
            scl = smp.tile([128, NB], F32, tag="scl")
            nc.vector.tensor_tensor(scl[:], rec[:], kout[:], OP.mult)
            for nb in range(NB):
                sl = slice(nb * 128, (nb + 1) * 128)
                ptf = psA.tile([128, COMP], BF16, tag="a", name="ptf")
                nc.tensor.transpose(ptf[:], fout[:, sl], w["idenc"][:COMP, :COMP])
                ot = outp.tile([128, COMP], F32, tag="ot")
                nc.scalar.activation(ot[:], ptf[:], AF.Tanh,
                                     scale=scl[:, nb : nb + 1])
                nc.sync.dma_start(out_d[sl, :], ot[:])

    return nc


_NC = None


def _get_nc():
    global _NC
    if _NC is None:
        _NC = build_nc()
        _NC.finalize()
    return _NC


def make_in_maps(inputs):
    w = host_prep(inputs)
    obs = np.ascontiguousarray(np.asarray(inputs["obs"], np.float32))
    act = np.ascontiguousarray(np.asarray(inputs["action"], np.float32))
    in_maps = []
    for i in range(NCORES):
        m = dict(w)
        m["obs"] = np.ascontiguousarray(obs[i * S:(i + 1) * S])
        m["action"] = np.ascontiguousarray(act[i * S:(i + 1) * S])
        in_maps.append(m)
    return in_maps


def kernel(**inputs):
    from concourse.bass_utils import run_bass_kernel_spmd

    nc = _get_nc()
    in_maps = make_in_maps(inputs)
    res = run_bass_kernel_spmd(nc, in_maps, core_ids=list(range(NCORES)))
    return np.concatenate([r["out"] for r in res.results], axis=0)
